# revision 1
# baseline (speedup 1.0000x reference)
"""Causal self-attention Bass kernel for Trainium2, 8 NeuronCores.

Problem shapes (hardcoded): x (4, 2048, 512), W_qkv (1536, 512),
W_out (512, 512), b_out (512,); NH=8 heads, DH=64.

Sharding: core c handles batch b = c // 2 and head group g = c % 2
(4 heads each). Each core computes its QKV slice, causal attention for
its 4 heads, and a partial output projection over its 256 y-dims.
Host sums the two partials per batch and adds the bias.

Per-core device pipeline (all matmul operands bf16, f32 PSUM accum):
  1. qkT projection: qkT[c_loc, t] chunks ordered [q-pair0, k-pair0,
     q-pair1, k-pair1] so each head's qT/kT share a 64-partition range.
  2. v projection directly in natural [t, d] layout, augmented with a
     ones column per head (denominator trick).
  3. Per head h, per key chunk j (128 rows): scoresT[k, q] for q >=
     128j only (causality via matmul geometry), one exp ACTIVATE per
     (h, j) with the 1/8 scale folded in, triangular mask applied to
     the 128x128 diagonal block post-exp (gpsimd multiply).
  4. att @ v with lhsT = [v | ones] (M=65): row 64 accumulates the
     softmax denominators for free.
  5. y normalization: partition-broadcast of the denominator row,
     vector reciprocal + multiply into the stacked ynormT layout.
  6. Output projection partial (K = 256 local y-dims).
"""

import sys

if "/opt/trn_rl_repo" not in sys.path:
    sys.path.insert(0, "/opt/trn_rl_repo")

import numpy as np
import ml_dtypes

B, T, D, NH, DH = 4, 2048, 512, 8, 64
HPC = 4  # heads per core
N_CORES = 8
BF16 = ml_dtypes.bfloat16

_PROG = None


def _build_program():
    import concourse.bass as bass
    import concourse.mybir as mybir
    import concourse.tile as tile
    from concourse import bacc

    f32 = mybir.dt.float32
    bf16 = mybir.dt.bfloat16
    Exp = mybir.ActivationFunctionType.Exp

    nc = bacc.Bacc("TRN2", target_bir_lowering=False, debug=False)

    xT_d = nc.dram_tensor("xT", [D, T], bf16, kind="ExternalInput").ap()
    wqkT_d = nc.dram_tensor("wqkT", [D, 512], bf16, kind="ExternalInput").ap()
    wvT_d = nc.dram_tensor("wvT", [D, 256], bf16, kind="ExternalInput").ap()
    woT_d = nc.dram_tensor("woT", [256, D], bf16, kind="ExternalInput").ap()
    triu_d = nc.dram_tensor("triu", [128, 128], bf16, kind="ExternalInput").ap()
    out_d = nc.dram_tensor("out", [T, D], f32, kind="ExternalOutput").ap()

    NT = T // 128  # 16 key/query 128-chunks
    NQ = T // 512  # 4 query 512-chunks

    with tile.TileContext(nc) as tc:
        with (
            tc.tile_pool(name="const", bufs=1) as cpool,
            tc.tile_pool(name="att", bufs=2) as apool,
            tc.tile_pool(name="work", bufs=2) as wpool,
            tc.tile_pool(name="outsb", bufs=2) as opool,
            tc.tile_pool(name="ps", bufs=1, space="PSUM") as pspool,
        ):
            # ---- input DMAs ----
            xT_sb = cpool.tile([128, 4, T], bf16, tag="xT")
            for kc in range(4):
                nc.sync.dma_start(out=xT_sb[:, kc, :], in_=xT_d[128 * kc : 128 * (kc + 1), :])
            wqkT_sb = cpool.tile([128, 4, 512], bf16, tag="wqkT")
            for kc in range(4):
                nc.sync.dma_start(out=wqkT_sb[:, kc, :], in_=wqkT_d[128 * kc : 128 * (kc + 1), :])
            wvT_sb = cpool.tile([128, 4, 256], bf16, tag="wvT")
            for kc in range(4):
                nc.sync.dma_start(out=wvT_sb[:, kc, :], in_=wvT_d[128 * kc : 128 * (kc + 1), :])
            woT_sb = cpool.tile([128, 2, 512], bf16, tag="woT")
            for kc in range(2):
                nc.sync.dma_start(out=woT_sb[:, kc, :], in_=woT_d[128 * kc : 128 * (kc + 1), :])
            triu_sb = cpool.tile([128, 128], bf16, tag="triu")
            nc.sync.dma_start(out=triu_sb[:, :], in_=triu_d[:, :])

            # qkT chunks: [q-pair0 | k-pair0 | q-pair1 | k-pair1]; head h at
            # partitions 64*(h%2) .. +64 of chunks (2*(h//2), 2*(h//2)+1).
            qkT_sb = cpool.tile([128, 4, T], bf16, tag="qkT")
            # v_all: per t-chunk, per head: 64 v-dims + a ones column (65).
            v_all = cpool.tile([128, NT, HPC * 65], bf16, tag="v_all")
            ynormT = cpool.tile([128, 2, T], bf16, tag="ynormT")

            ones_ap = v_all[:, :, :].rearrange("p t (h e) -> p (t h) e", e=65)[:, :, 64:65]
            nc.gpsimd.memset(ones_ap, 1.0)
            # ones row at partition 64 used to broadcast the denominator row
            # via a rank-1 fp32 matmul (walrus can't codegen
            # InstPartitionBroadcast; matmul wants lhsT/rhs on the same base
            # partition as the den row, and fp32r inputs would need rounding)
            ones64 = cpool.tile([65, 64], f32, tag="ones64")
            nc.gpsimd.memset(ones64[:, :], 1.0)

            # ---- phase A: projections (interleave qk chunks with v halves
            # so the PSUM slots alternate and DVE copies hide under PE) ----
            def emit_proj_m(m):
                for half in range(2):
                    ps = pspool.tile([128, 1024], f32, tag="ps_s", bufs=2)
                    for n2 in range(2):
                        n = 2 * half + n2
                        for kc in range(4):
                            nc.tensor.matmul(
                                ps[:, 512 * n2 : 512 * (n2 + 1)],
                                lhsT=wqkT_sb[:, kc, 128 * m : 128 * (m + 1)],
                                rhs=xT_sb[:, kc, 512 * n : 512 * (n + 1)],
                                start=(kc == 0),
                                stop=(kc == 3),
                            )
                    nc.vector.tensor_copy(
                        qkT_sb[:, m, 1024 * half : 1024 * (half + 1)], ps[:, :]
                    )

            def emit_proj_v(half):
                ps = pspool.tile([128, T], f32, tag="ps_y")
                for tl in range(8):
                    t = 8 * half + tl
                    for kc in range(4):
                        nc.tensor.matmul(
                            ps[:, 256 * tl : 256 * (tl + 1)],
                            lhsT=xT_sb[:, kc, 128 * t : 128 * (t + 1)],
                            rhs=wvT_sb[:, kc, :],
                            start=(kc == 0),
                            stop=(kc == 3),
                        )
                dst = v_all[:, 8 * half : 8 * (half + 1), :].rearrange(
                    "p t (h e) -> p t h e", e=65
                )[:, :, :, 0:64]
                src = ps[:, :].rearrange("p (t h e) -> p t h e", t=8, h=HPC)
                nc.vector.tensor_copy(dst, src)

            emit_proj_m(0)
            emit_proj_v(0)
            emit_proj_m(1)
            emit_proj_v(1)
            emit_proj_m(2)
            emit_proj_m(3)

            # ---- phase B/C: per-head attention ----
            def emit_av(h, j, ps_y, att):
                # accumulate yT_aug[:, q] += v_augT @ attT for key chunk j
                q0 = 128 * j
                lhsT = v_all[:, j, :].rearrange("p (h e) -> p h e", e=65)[:, h, :]
                for c in range(NQ):
                    if j > 4 * c + 3:
                        continue  # fully masked: k > all q in this chunk
                    qs = max(512 * c, q0)
                    qe = 512 * (c + 1)
                    nc.tensor.matmul(
                        ps_y[:, qs:qe],
                        lhsT=lhsT,
                        rhs=att[:, qs - q0 : qe - q0],
                        start=(j == 0),
                        stop=(j == 4 * c + 3),
                    )

            for h in range(HPC):
                base = 64 * (h % 2)
                qc = 2 * (h // 2)  # q chunk index; k chunk is qc + 1
                qT = qkT_sb[base : base + 64, qc, :]
                kT = qkT_sb[base : base + 64, qc + 1, :]
                ps_y = pspool.tile([65, T], f32, tag="ps_y")
                att_tiles = {}
                for j in range(NT):
                    q0 = 128 * j
                    att = apool.tile([128, T - q0], bf16, tag=f"att{j}")
                    # split the q range at 1024 so scores(j+1) can run in the
                    # second ps_s buffer while exp(j) drains the first
                    halves = [(q0, 1024), (1024, T)] if j < 8 else [(q0, T)]
                    for hs, he in halves:
                        ps_s = pspool.tile([128, 1024], f32, tag="ps_s", bufs=2)
                        for rel in range(0, he - hs, 512):
                            n = min(512, he - hs - rel)
                            nc.tensor.matmul(
                                ps_s[:, rel : rel + n],
                                lhsT=kT[:, 128 * j : 128 * (j + 1)],
                                rhs=qT[:, hs + rel : hs + rel + n],
                                start=True,
                                stop=True,
                            )
                        nc.scalar.activation(
                            att[:, hs - q0 : he - q0],
                            ps_s[:, 0 : he - hs],
                            Exp,
                            scale=0.125,
                        )
                    # mask the strict upper triangle of the diagonal block
                    nc.gpsimd.tensor_mul(att[:, 0:128], att[:, 0:128], triu_sb[:, :])
                    att_tiles[j] = att
                    if j >= 1:
                        emit_av(h, j - 1, ps_y, att_tiles[j - 1])
                emit_av(h, NT - 1, ps_y, att_tiles[NT - 1])

                # normalization: y / den, written to stacked ynormT
                yT = wpool.tile([65, T], f32, tag="yT")
                nc.vector.tensor_copy(yT[:, :], ps_y[:, :])
                # broadcast den row across 64 partitions: ones[1,64].T @ den
                ps_b = pspool.tile([64, T], f32, tag="ps_y")
                for c in range(NQ):
                    nc.tensor.matmul(
                        ps_b[:, 512 * c : 512 * (c + 1)],
                        lhsT=ones64[64:65, :],
                        rhs=yT[64:65, 512 * c : 512 * (c + 1)],
                        start=True,
                        stop=True,
                    )
                recb = wpool.tile([64, T], f32, tag="recb")
                nc.vector.reciprocal_approx_fast(out=recb[:, :], in_=ps_b[:, :])
                dst = ynormT[base : base + 64, h // 2, :]
                nc.vector.tensor_mul(dst, yT[0:64, :], recb[:, :])

            # ---- phase D: output projection partial ----
            for grp in range(8):
                ps = pspool.tile([128, 1024], f32, tag="ps_s", bufs=2)
                for i2 in range(2):
                    i = 2 * grp + i2
                    for kc in range(2):
                        nc.tensor.matmul(
                            ps[:, 512 * i2 : 512 * (i2 + 1)],
                            lhsT=ynormT[:, kc, 128 * i : 128 * (i + 1)],
                            rhs=woT_sb[:, kc, :],
                            start=(kc == 0),
                            stop=(kc == 1),
                        )
                osb = opool.tile([128, 1024], f32, tag="osb")
                nc.vector.tensor_copy(osb[:, :], ps[:, :])
                for i2 in range(2):
                    i = 2 * grp + i2
                    nc.sync.dma_start(
                        out=out_d[128 * i : 128 * (i + 1), :],
                        in_=osb[:, 512 * i2 : 512 * (i2 + 1)],
                    )

    nc.compile()
    return nc


def _get_program():
    global _PROG
    if _PROG is None:
        _PROG = _build_program()
    return _PROG


def _make_in_maps(x, W_qkv, W_out):
    in_maps = []
    triu = np.triu(np.ones((128, 128), np.float32)).astype(BF16)
    for c in range(N_CORES):
        b, g = c // 2, c % 2
        heads = [4 * g + i for i in range(HPC)]
        xT = np.ascontiguousarray(x[b].T).astype(BF16)
        # qkT chunk order: [q-pair0, k-pair0, q-pair1, k-pair1]
        rows = []
        for hp in range(2):
            h0, h1 = heads[2 * hp], heads[2 * hp + 1]
            rows.append(W_qkv[64 * h0 : 64 * h0 + 64])  # q of h0
            rows.append(W_qkv[64 * h1 : 64 * h1 + 64])  # q of h1
            rows.append(W_qkv[512 + 64 * h0 : 512 + 64 * h0 + 64])  # k of h0
            rows.append(W_qkv[512 + 64 * h1 : 512 + 64 * h1 + 64])  # k of h1
        W_perm = np.concatenate(rows, axis=0)  # (512, 512)
        wqkT = np.ascontiguousarray(W_perm.T).astype(BF16)
        wv = W_qkv[1024 + 256 * g : 1024 + 256 * (g + 1)]  # (256, 512)
        wvT = np.ascontiguousarray(wv.T).astype(BF16)
        wo = W_out[:, 256 * g : 256 * (g + 1)]  # (512, 256)
        woT = np.ascontiguousarray(wo.T).astype(BF16)
        in_maps.append(
            {"xT": xT, "wqkT": wqkT, "wvT": wvT, "woT": woT, "triu": triu}
        )
    return in_maps


def kernel(x, W_qkv, W_out, b_out):
    from concourse.bass_utils import run_bass_kernel_spmd

    x = np.asarray(x, np.float32)
    W_qkv = np.asarray(W_qkv, np.float32)
    W_out = np.asarray(W_out, np.float32)
    b_out = np.asarray(b_out, np.float32)

    nc = _get_program()
    in_maps = _make_in_maps(x, W_qkv, W_out)
    res = run_bass_kernel_spmd(nc, in_maps, list(range(N_CORES)))
    outs = [r["out"].astype(np.float32) for r in res.results]
    full = np.empty((B, T, D), np.float32)
    for b in range(B):
        full[b] = outs[2 * b] + outs[2 * b + 1] + b_out
    return full



# revision 4
# speedup vs baseline: 1.1658x; 1.1658x over previous
"""Causal self-attention Bass kernel for Trainium2, 8 NeuronCores.

Problem shapes (hardcoded): x (4, 2048, 512), W_qkv (1536, 512),
W_out (512, 512), b_out (512,); NH=8 heads, DH=64.

Sharding: core c handles batch b = c // 2 and head group g = c % 2
(4 heads each). Each core computes its QKV slice, causal attention for
its 4 heads, and a partial output projection over its 256 y-dims.
Host sums the two partials per batch and adds the bias.

Per-core device pipeline (all matmul operands bf16, f32 PSUM accum):
  1. Fine-grained input DMAs ordered so the first projection matmul's
     deps (wqkT m-pair 0, xT token-half 0) land first.
  2. qkT projection per (m, token-half); v projection per t-quartet.
     All projection/score/output matmuls share one [128, 1024] PSUM
     pool (2 bufs) so PSUM stays within 8 banks.
  3. Per head h, per key chunk j: scoresT[k, q] for q >= 128j, exp
     ACTIVATE with 1/8 scale folded in, triangular mask on the
     diagonal 128x128 block via DVE multiply (post-exp).
  4. att @ v accumulated per 512-col query quarter into [65, 512]
     PSUM tiles (v augmented with a ones column -> row 64 is the
     softmax denominator). Quarters drain as soon as their last key
     chunk lands, spreading normalization work and freeing PSUM.
  5. Normalization per quarter: den row copied to bf16 (gpsimd),
     broadcast across 64 partitions via a rank-1 bf16 matmul (cheap,
     vs 4 cyc/row for f32), DVE reciprocal + multiply into ynormT.
  6. Output projection partial (K = 256 local y-dims) + per-group DMA.
Projections for later heads are woven into head 0/1's score stream to
keep the PE dense while the Scalar engine (exp) is the bottleneck.
"""

import sys

if "/opt/trn_rl_repo" not in sys.path:
    sys.path.insert(0, "/opt/trn_rl_repo")

import numpy as np
import ml_dtypes

B, T, D, NH, DH = 4, 2048, 512, 8, 64
HPC = 4  # heads per core
N_CORES = 8
BF16 = ml_dtypes.bfloat16

_PROG = None


def _build_program():
    import concourse.bass as bass
    import concourse.mybir as mybir
    import concourse.tile as tile
    from concourse import bacc

    f32 = mybir.dt.float32
    bf16 = mybir.dt.bfloat16
    Exp = mybir.ActivationFunctionType.Exp

    nc = bacc.Bacc("TRN2", target_bir_lowering=False, debug=False)

    xT_d = nc.dram_tensor("xT", [D, T], bf16, kind="ExternalInput").ap()
    wqkT_d = nc.dram_tensor("wqkT", [D, 512], bf16, kind="ExternalInput").ap()
    wvT_d = nc.dram_tensor("wvT", [D, 256], bf16, kind="ExternalInput").ap()
    woT_d = nc.dram_tensor("woT", [256, D], bf16, kind="ExternalInput").ap()
    triu_d = nc.dram_tensor("triu", [128, 128], bf16, kind="ExternalInput").ap()
    out_d = nc.dram_tensor("out", [T, D], f32, kind="ExternalOutput").ap()

    NT = T // 128  # 16 key/query 128-chunks
    NQ = T // 512  # 4 query 512-chunks

    with tile.TileContext(nc) as tc:
        with (
            tc.tile_pool(name="const", bufs=1) as cpool,
            tc.tile_pool(name="att", bufs=1) as apool,
            tc.tile_pool(name="work", bufs=2) as wpool,
            tc.tile_pool(name="outsb", bufs=2) as opool,
            tc.tile_pool(name="ps", bufs=1, space="PSUM") as pspool,
        ):
            # ---- input DMAs, ordered by first use ----
            xT_sb = cpool.tile([128, 4, T], bf16, tag="xT")
            wqkT_sb = cpool.tile([128, 4, 512], bf16, tag="wqkT")
            wvT_sb = cpool.tile([128, 4, 256], bf16, tag="wvT")
            woT_sb = cpool.tile([128, 2, 512], bf16, tag="woT")
            triu_sb = cpool.tile([128, 128], bf16, tag="triu")

            for kc in range(4):  # wqkT m-pair 0 (chunks m=0,1)
                nc.sync.dma_start(
                    out=wqkT_sb[:, kc, 0:256],
                    in_=wqkT_d[128 * kc : 128 * (kc + 1), 0:256],
                )
            for th in range(2):  # xT token halves
                for kc in range(4):
                    nc.sync.dma_start(
                        out=xT_sb[:, kc, 1024 * th : 1024 * (th + 1)],
                        in_=xT_d[128 * kc : 128 * (kc + 1), 1024 * th : 1024 * (th + 1)],
                    )
            for kc in range(4):
                nc.sync.dma_start(out=wvT_sb[:, kc, :], in_=wvT_d[128 * kc : 128 * (kc + 1), :])
            nc.sync.dma_start(out=triu_sb[:, :], in_=triu_d[:, :])
            for kc in range(4):  # wqkT m-pair 1 (chunks m=2,3)
                nc.sync.dma_start(
                    out=wqkT_sb[:, kc, 256:512],
                    in_=wqkT_d[128 * kc : 128 * (kc + 1), 256:512],
                )
            for kc in range(2):
                nc.sync.dma_start(out=woT_sb[:, kc, :], in_=woT_d[128 * kc : 128 * (kc + 1), :])

            # qkT chunks: [q-pair0 | k-pair0 | q-pair1 | k-pair1]; head h at
            # partitions 64*(h%2) .. +64 of chunks (2*(h//2), 2*(h//2)+1).
            qkT_sb = cpool.tile([128, 4, T], bf16, tag="qkT")
            # v_all: per t-chunk, per head: 64 v-dims + a ones column (65).
            v_all = cpool.tile([128, NT, HPC * 65], bf16, tag="v_all")
            ynormT = cpool.tile([128, 2, T], bf16, tag="ynormT")

            ones_ap = v_all[:, :, :].rearrange("p t (h e) -> p (t h) e", e=65)[:, :, 64:65]
            nc.gpsimd.memset(ones_ap, 1.0)
            # bf16 ones row at partition 64: broadcasts the denominator row
            # across 64 partitions via a rank-1 bf16 matmul (1 cyc/row vs 4
            # for f32; walrus can't codegen InstPartitionBroadcast).
            onesb = cpool.tile([65, 64], bf16, tag="onesb")
            nc.gpsimd.memset(onesb[:, :], 1.0)

            # ---- emission helpers ----
            def emit_proj_m(m, half):
                # qkT[:, m, 1024*half : +1024] for token half `half`
                ps = pspool.tile([128, 1024], f32, tag="ps", bufs=2)
                for n2 in range(2):
                    n = 2 * half + n2
                    for kc in range(4):
                        nc.tensor.matmul(
                            ps[:, 512 * n2 : 512 * (n2 + 1)],
                            lhsT=wqkT_sb[:, kc, 128 * m : 128 * (m + 1)],
                            rhs=xT_sb[:, kc, 512 * n : 512 * (n + 1)],
                            start=(kc == 0),
                            stop=(kc == 3),
                        )
                nc.vector.tensor_copy(
                    qkT_sb[:, m, 1024 * half : 1024 * (half + 1)], ps[:, :]
                )

            def emit_proj_v(q):
                # v for t-chunks 4q .. 4q+3 (all 4 heads)
                ps = pspool.tile([128, 1024], f32, tag="ps", bufs=2)
                for tl in range(4):
                    t = 4 * q + tl
                    for kc in range(4):
                        nc.tensor.matmul(
                            ps[:, 256 * tl : 256 * (tl + 1)],
                            lhsT=xT_sb[:, kc, 128 * t : 128 * (t + 1)],
                            rhs=wvT_sb[:, kc, :],
                            start=(kc == 0),
                            stop=(kc == 3),
                        )
                dst = v_all[:, 4 * q : 4 * (q + 1), :].rearrange(
                    "p t (h e) -> p t h e", e=65
                )[:, :, :, 0:64]
                src = ps[:, :].rearrange("p (t h e) -> p t h e", t=4, h=HPC)
                nc.vector.tensor_copy(dst, src)

            def emit_scores(h, j, att):
                base = 64 * (h % 2)
                qc = 2 * (h // 2)
                qT = qkT_sb[base : base + 64, qc, :]
                kT = qkT_sb[base : base + 64, qc + 1, :]
                q0 = 128 * j
                halves = [(q0, 1024), (1024, T)] if j < 8 else [(q0, T)]
                for hs, he in halves:
                    ps_s = pspool.tile([128, 1024], f32, tag="ps", bufs=2)
                    for rel in range(0, he - hs, 512):
                        n = min(512, he - hs - rel)
                        nc.tensor.matmul(
                            ps_s[:, rel : rel + n],
                            lhsT=kT[:, q0 : q0 + 128],
                            rhs=qT[:, hs + rel : hs + rel + n],
                            start=True,
                            stop=True,
                        )
                    nc.scalar.activation(
                        att[:, hs - q0 : he - q0],
                        ps_s[:, 0 : he - hs],
                        Exp,
                        scale=0.125,
                    )
                # mask the strict upper triangle of the diagonal block
                nc.vector.tensor_mul(att[:, 0:128], att[:, 0:128], triu_sb[:, :])

            def emit_av(h, c, att_tiles):
                # accumulate y quarter c (q cols 512c..512c+512) over key
                # chunks j = 0 .. 4c+3; row 64 = softmax denominator.
                yq = pspool.tile([65, 512], f32, tag="yq", bufs=3)
                for j in range(4 * c + 4):
                    lhsT = v_all[:, j, :].rearrange("p (h e) -> p h e", e=65)[:, h, :]
                    rel0 = 512 * c - 128 * j
                    nc.tensor.matmul(
                        yq[:, max(0, -rel0) : 512],
                        lhsT=lhsT,
                        rhs=att_tiles[j][:, max(0, rel0) : rel0 + 512],
                        start=(j == 0),
                        stop=(j == 4 * c + 3),
                    )
                # den row -> bf16 (gpsimd can't read PSUM, so DVE)
                denb = wpool.tile([65, 512], bf16, tag="denb")
                nc.vector.tensor_copy(denb[64:65, :], yq[64:65, :])
                return yq, denb

            def emit_norm_bcast(state):
                # rank-1 bf16 matmul: broadcast den row across 64 partitions
                h, c, yq, denb = state
                ps_b = pspool.tile([64, 512], f32, tag="ps_b", bufs=1)
                nc.tensor.matmul(
                    ps_b[:, :],
                    lhsT=onesb[64:65, :],
                    rhs=denb[64:65, :],
                    start=True,
                    stop=True,
                )
                recb = wpool.tile([64, 512], f32, tag="recb")
                nc.vector.reciprocal_approx_fast(out=recb[:, :], in_=ps_b[:, :])
                base = 64 * (h % 2)
                dst = ynormT[base : base + 64, h // 2, 512 * c : 512 * (c + 1)]
                nc.vector.tensor_mul(dst, yq[0:64, :], recb[:, :])

            # ---- phase A prefix: enough for heads 0/1 to start ----
            emit_proj_m(0, 0)
            emit_proj_m(1, 0)
            emit_proj_m(0, 1)
            emit_proj_m(1, 1)
            emit_proj_v(0)
            emit_proj_v(1)

            # ---- phases B/C per head, remaining projections woven in ----
            # weave[h][i]: emitted after scores chunk group i of head h
            weave = {
                0: {4: [lambda: emit_proj_v(2)], 8: [lambda: emit_proj_v(3)]},
                1: {
                    4: [lambda: emit_proj_m(2, 0), lambda: emit_proj_m(2, 1)],
                    8: [lambda: emit_proj_m(3, 0), lambda: emit_proj_m(3, 1)],
                },
            }
            for h in range(HPC):
                att_tiles = {}
                pending = None  # quarter awaiting its broadcast matmul
                for j in range(NT):
                    att = apool.tile([128, T - 128 * j], bf16, tag=f"att{h % 2}_{j}")
                    emit_scores(h, j, att)
                    att_tiles[j] = att
                    for fn in weave.get(h, {}).get(j, []):
                        fn()
                    if j % 4 == 3:
                        c = j // 4
                        if pending is not None:
                            emit_norm_bcast(pending)
                        yq, denb = emit_av(h, c, att_tiles)
                        pending = (h, c, yq, denb)
                emit_norm_bcast(pending)

            # ---- phase D: output projection partial ----
            for grp in range(8):
                ps = pspool.tile([128, 1024], f32, tag="ps", bufs=2)
                for i2 in range(2):
                    i = 2 * grp + i2
                    for kc in range(2):
                        nc.tensor.matmul(
                            ps[:, 512 * i2 : 512 * (i2 + 1)],
                            lhsT=ynormT[:, kc, 128 * i : 128 * (i + 1)],
                            rhs=woT_sb[:, kc, :],
                            start=(kc == 0),
                            stop=(kc == 1),
                        )
                osb = opool.tile([128, 1024], f32, tag="osb")
                nc.vector.tensor_copy(osb[:, :], ps[:, :])
                for i2 in range(2):
                    i = 2 * grp + i2
                    nc.sync.dma_start(
                        out=out_d[128 * i : 128 * (i + 1), :],
                        in_=osb[:, 512 * i2 : 512 * (i2 + 1)],
                    )

    nc.compile()
    return nc


def _get_program():
    global _PROG
    if _PROG is None:
        _PROG = _build_program()
    return _PROG


def _make_in_maps(x, W_qkv, W_out):
    in_maps = []
    triu = np.triu(np.ones((128, 128), np.float32)).astype(BF16)
    for c in range(N_CORES):
        b, g = c // 2, c % 2
        heads = [4 * g + i for i in range(HPC)]
        xT = np.ascontiguousarray(x[b].T).astype(BF16)
        # qkT chunk order: [q-pair0, k-pair0, q-pair1, k-pair1]
        rows = []
        for hp in range(2):
            h0, h1 = heads[2 * hp], heads[2 * hp + 1]
            rows.append(W_qkv[64 * h0 : 64 * h0 + 64])  # q of h0
            rows.append(W_qkv[64 * h1 : 64 * h1 + 64])  # q of h1
            rows.append(W_qkv[512 + 64 * h0 : 512 + 64 * h0 + 64])  # k of h0
            rows.append(W_qkv[512 + 64 * h1 : 512 + 64 * h1 + 64])  # k of h1
        W_perm = np.concatenate(rows, axis=0)  # (512, 512)
        wqkT = np.ascontiguousarray(W_perm.T).astype(BF16)
        wv = W_qkv[1024 + 256 * g : 1024 + 256 * (g + 1)]  # (256, 512)
        wvT = np.ascontiguousarray(wv.T).astype(BF16)
        wo = W_out[:, 256 * g : 256 * (g + 1)]  # (512, 256)
        woT = np.ascontiguousarray(wo.T).astype(BF16)
        in_maps.append(
            {"xT": xT, "wqkT": wqkT, "wvT": wvT, "woT": woT, "triu": triu}
        )
    return in_maps


def kernel(x, W_qkv, W_out, b_out):
    from concourse.bass_utils import run_bass_kernel_spmd

    x = np.asarray(x, np.float32)
    W_qkv = np.asarray(W_qkv, np.float32)
    W_out = np.asarray(W_out, np.float32)
    b_out = np.asarray(b_out, np.float32)

    nc = _get_program()
    in_maps = _make_in_maps(x, W_qkv, W_out)
    res = run_bass_kernel_spmd(nc, in_maps, list(range(N_CORES)))
    outs = [r["out"].astype(np.float32) for r in res.results]
    full = np.empty((B, T, D), np.float32)
    for b in range(B):
        full[b] = outs[2 * b] + outs[2 * b + 1] + b_out
    return full
